# revision 47
# baseline (speedup 1.0000x reference)
"""Additive (Bahdanau) attention on 8 Trainium2 NeuronCores.

Reference math (per batch b):
    qh = queries @ Wq                  (NQ, H)
    kh = keys    @ Wk                  (NK, H)
    scores[q,k] = sum_h wv[h] * tanh(qh[q,h] + kh[k,h])
    attn = softmax(mask(scores))       mask: k >= valid_len -> -1e6
    out  = attn @ values               (NQ, V)

Sharding (flash-style, valid-length aware): masked keys contribute exactly
zero to the softmax (the reference's exp(-1e6 - max) underflows to 0.0), so
only k < valid_len needs computing. The valid (batch, q-half, k-chunk) space
is split into uniform work tiles of (128 q-rows x 512 keys); tiles are
distributed round-robin over the 8 cores (padded with zero-mask dummy tiles
to a multiple of 8, T = tiles-per-core is 1..4). Every core runs the same
SPMD graph over T tiles. Each tile emits the UNNORMALIZED partials
(sum_k p*V | sum_k p) as a (128, 65) block; the host sums partials of the
same (batch, q-half) across tiles and divides - the cross-shard softmax
renormalization. No max-subtraction is needed: |scores| <= ||wv||_1 (~5),
so exp never overflows, and the missing shift cancels in the p/l ratio.
Math is exact up to rounding; bf16 matmul inputs with fp32 PSUM
accumulation give ~3e-3 relative error on the final output.

Per-tile device pipeline (q=128 -> 32 groups of 4, k=512):
  - partitions carry (j, h) = (q mod 4, h) -> 4*32 = 128 lanes
  - kh4 (kh replicated 4x over partition groups) via one col-tiled matmul
    set into a 1-bank psum tile, narrowed to bf16 in SBUF
  - qh4[(j,h), g] = qh[4g+j, h] via 4 col-tiled matmuls
  - loop over q-group chunks (2,2,4,8,8,8): DVE adds the per-group bias
    (per-partition scalar), ScalarE runs one big in-place tanh per chunk,
    TensorE reduces over h with zero-padded (128, 32) stationary weights
    (M=32 supergroup col-tiling) accumulating scores in psum
  - P = exp(scores) from psum; PE transposes P (4 tiles of 128x128);
    DVE multiplies by the 0/1 mask column during the psum->sbuf copy;
    accumulate [V | 1] matmuls into the (128, 65) partial output
Successive tiles pipeline: tile t+1's tanh stream runs while tile t's
softmax tail finishes, so only the last tile's tail is exposed.
"""

import ml_dtypes
import numpy as np

import concourse.bacc as bacc
import concourse.tile as tile
from concourse import mybir
from concourse.bass_utils import run_bass_kernel_spmd

B, NQ, NK = 4, 256, 2048
QKD, H, VD = 64, 32, 64
NQS = 128          # q rows per tile
NG = NQS // 4      # 32 q-groups of 4
KC = 512           # keys per tile
KT = KC // 128     # 4 k-subtiles per tile
CHUNKS = [2, 2, 4, 8, 8, 8]
CHUNKS_LAST = [2, 2, 4, 8, 8, 4, 2, 1, 1]
F32 = mybir.dt.float32
BF16 = mybir.dt.bfloat16

_cache = {}


def _build_nc(T):
    """Build the SPMD graph processing T work tiles per core."""
    nc = bacc.Bacc("TRN2", debug=False, num_devices=8,
                   monotonic_sem_count=0, enable_asserts=False,
                   num_swdge_queues=4)

    # blob columns: [0:32]=wq, [32:64]=wk, [64:64+128T]=qTr per tile,
    # [64+128T:192+128T]=ident, then 4T mask columns
    BW = 192 + 132 * T
    d_kT = nc.declare_dram_parameter("kT", [QKD, KC * T], BF16, isOutput=False)
    d_blob = nc.declare_dram_parameter("blob", [128, BW], BF16, isOutput=False)
    d_wvb = nc.declare_dram_parameter("wvb", [128, NG * 32], BF16, isOutput=False)
    d_vaug = nc.declare_dram_parameter("vaug", [128, KT * 65 * T], BF16,
                                       isOutput=False)
    d_out = nc.declare_dram_parameter("out", [NQS, 65 * T], F32, isOutput=True)

    TANH = mybir.ActivationFunctionType.Tanh
    EXP = mybir.ActivationFunctionType.Exp

    with tile.TileContext(nc) as tc:
        with (
            tc.tile_pool(name="sb", bufs=1) as sb,
            tc.tile_pool(name="fpool", bufs=2) as fpool,
            tc.tile_pool(name="psA", bufs=1, space="PSUM") as psA,
            tc.tile_pool(name="psB", bufs=1, space="PSUM") as psB,
        ):
            kT_sb = sb.tile([QKD, KC * T], BF16, tag="kT")
            blob_sb = sb.tile([128, BW], BF16, tag="blob")
            wvb_sb = sb.tile([128, NG * 32], BF16, tag="wvb")
            vaug_sb = sb.tile([128, KT * 65 * T], BF16, tag="vaug")
            qh4_sb = sb.tile([128, NG * T], F32, tag="qh4")
            kh4bf_sb = sb.tile([128, KC * T], BF16, tag="kh4bf")
            wq_sb = blob_sb[0:QKD, 0:32]
            wk_sb = blob_sb[0:QKD, 32:64]
            qTr_all = blob_sb[0:QKD, 64:64 + 128 * T]
            ident_sb = blob_sb[:, 64 + 128 * T:192 + 128 * T]
            maskc_bf = blob_sb[:, 192 + 128 * T:BW]
            maskc_sb = sb.tile([128, 4 * T], F32, tag="maskf")
            out_sb = sb.tile([NQS, 65 * T], F32, tag="outsb")
            P_sb = sb.tile([128, KC * T], BF16, tag="P")
            PT_sb = sb.tile([128, KC * T], BF16, tag="PT")

            # split the early DMAs across engine queues
            nc.sync.dma_start(out=kT_sb[:, 0:KC], in_=d_kT[:, 0:KC])
            if T > 1:
                nc.scalar.dma_start(out=kT_sb[:, KC:], in_=d_kT[:, KC:])
            nc.gpsimd.dma_start(out=blob_sb[:], in_=d_blob[:])

            qh4_ps = psB.tile([128, NG * T], F32, tag="sc0")
            for t in range(T):
                for j in range(4):
                    nc.tensor.matmul(
                        qh4_ps[32 * j:32 * (j + 1), t * NG:(t + 1) * NG],
                        lhsT=wq_sb,
                        rhs=qTr_all[:, t * 128 + j * 32:t * 128 + (j + 1) * 32],
                        start=True, stop=True,
                        tile_position=(0, 32 * j),
                    )
            nc.scalar.copy(qh4_sb[:], qh4_ps[:])

            # per-tile kh4 psum (1 bank each) -> bf16 sbuf
            kh4c = [psA.tile([128, KC], F32, tag=f"kh{t}", name=f"kh4c{t}")
                    for t in range(T)]
            for t in range(T):
                for j in range(4):
                    nc.tensor.matmul(
                        kh4c[t][32 * j:32 * (j + 1), :],
                        lhsT=wk_sb,
                        rhs=kT_sb[:, t * KC:(t + 1) * KC],
                        start=True, stop=True,
                        tile_position=(0, 32 * j),
                    )
                # alternate cast engines so they pipeline
                cp = nc.scalar.copy if t % 2 == 0 else nc.vector.tensor_copy
                cp(kh4bf_sb[:, t * KC:(t + 1) * KC], kh4c[t][:])

            nc.vector.tensor_copy(maskc_sb[:], maskc_bf)
            scores = [psB.tile([128, KC], F32, tag=f"sc{t}", name=f"sc{t}")
                      for t in range(T)]
            PTb = [psA.tile([128, 2 * KC], BF16, tag=f"kh{t}", name=f"PTb{t}")
                   for t in range(T)]
            av = [psB.tile([128, 65], F32, tag=f"sc{t}", name=f"av{t}")
                  for t in range(T)]

            def score_mm(t, gg, rhs):
                G = gg // 8
                nc.tensor.matmul(
                    scores[t][32 * G:32 * (G + 1), :],
                    lhsT=wvb_sb[:, gg * 32:(gg + 1) * 32],
                    rhs=rhs,
                    start=(gg % 8 == 0), stop=(gg % 8 == 7),
                    skip_group_check=True,
                    tile_position=(0, 32 * G),
                )

            for t in range(T):
                # ---- tanh + h-reduction over this tile's 512 keys ----
                g = 0
                if t == 0:
                    # bridge the cast+add startup latency: first two groups
                    # tanh straight from the kh4 psum with a per-group bias
                    for gg in range(2):
                        Fb = fpool.tile([128, KC], BF16, tag="Fs1",
                                        bufs=4, name=f"Fb_{gg}")
                        nc.scalar.activation(
                            Fb[:], kh4c[0][:], TANH,
                            bias=qh4_sb[:, gg:gg + 1], scale=1.0,
                        )
                        if gg == 0:
                            nc.scalar.dma_start(out=wvb_sb[:], in_=d_wvb[:])
                            nc.gpsimd.dma_start(out=vaug_sb[:], in_=d_vaug[:])
                        score_mm(0, gg, Fb[:])
                    g = 2
                chunks = CHUNKS_LAST if t == T - 1 else CHUNKS
                if t == 0:
                    chunks = [2, 4, 8, 8, 8] if T > 1 else [4, 8, 8, 4, 2, 2, 1, 1]
                for nch in chunks:
                    Fs = fpool.tile([128, nch * KC], BF16, tag=f"Fs{nch}",
                                    bufs={1: 4, 2: 4, 4: 3, 8: 3, 16: 2}[nch],
                                    name=f"Fs_{t}_{g}")
                    for i in range(nch):
                        nc.vector.tensor_scalar_add(
                            Fs[:, i * KC:(i + 1) * KC],
                            kh4bf_sb[:, t * KC:(t + 1) * KC],
                            qh4_sb[:, t * NG + g + i:t * NG + g + i + 1],
                        )
                    nc.scalar.activation(Fs[:], Fs[:], TANH)
                    for i in range(nch):
                        score_mm(t, g + i, Fs[:, i * KC:(i + 1) * KC])
                    g += nch

                # ---- softmax numerator + masked AV partials ----
                nc.scalar.activation(
                    P_sb[:, t * KC:(t + 1) * KC], scores[t][:], EXP)
                for s in range(KT):
                    pcol = t * KC + s * 128
                    off = (s % 2) * 512 + (s // 2) * 128
                    pt = PTb[t][:, off:off + 128]
                    nc.tensor.transpose(
                        pt, P_sb[:, pcol:pcol + 128], ident_sb)
                    nc.vector.tensor_scalar_mul(
                        PT_sb[:, pcol:pcol + 128], pt,
                        maskc_sb[:, t * KT + s:t * KT + s + 1],
                    )
                    nc.tensor.matmul(
                        av[t][:],
                        lhsT=PT_sb[:, pcol:pcol + 128],
                        rhs=vaug_sb[:, (t * KT + s) * 65:(t * KT + s + 1) * 65],
                        start=(s == 0), stop=(s == KT - 1),
                    )
                nc.vector.tensor_copy(
                    out_sb[:, t * 65:(t + 1) * 65], av[t][:])
                nc.sync.dma_start(
                    out=d_out[:, t * 65:(t + 1) * 65],
                    in_=out_sb[:, t * 65:(t + 1) * 65])

    nc.compile()
    return nc


def _host_shards(queries, keys, values, valid_lens, Wq, Wk, wv):
    """Build the balanced valid-key tile assignment and per-core inputs.
    Host work is layout/marshaling only; all tensor FLOPs run on device."""
    f32 = np.float32
    bf16 = ml_dtypes.bfloat16
    queries = np.asarray(queries, f32)
    keys = np.asarray(keys, f32)
    values = np.asarray(values, f32)
    valid_lens = np.asarray(valid_lens)
    Wq = np.asarray(Wq, f32)
    Wk = np.asarray(Wk, f32)
    wv = np.asarray(wv, f32)

    # work tiles: (batch, q-half, k-chunk) over the valid key range
    tiles = []
    for b in range(B):
        nk_chunks = max(1, int(np.ceil(int(valid_lens[b]) / KC)))
        for half in range(NQ // NQS):
            for kc in range(nk_chunks):
                tiles.append((b, half, kc))
    while len(tiles) % 8 != 0:
        tiles.append(None)                     # zero-mask dummy
    T = len(tiles) // 8

    # zero-padded stationary weights (M=32 supergroup col-tiling)
    wvb = np.zeros((128, NG * 32), f32)
    for g in range(NG):
        for j in range(4):
            wvb[j * 32:(j + 1) * 32, g * 32 + 4 * (g % 8) + j] = wv

    BW = 192 + 132 * T
    blob_base = np.zeros((128, BW), f32)
    blob_base[0:QKD, 0:32] = Wq
    blob_base[0:QKD, 32:64] = Wk
    blob_base[:, 64 + 128 * T:192 + 128 * T] = np.eye(128, dtype=f32)
    shared = {"wvb": wvb.astype(bf16)}

    assign = [tiles[c::8] for c in range(8)]   # round-robin -> balanced
    in_maps = []
    for core in range(8):
        kT = np.zeros((QKD, KC * T), f32)
        vaug = np.zeros((128, KT * 65 * T), f32)
        blob = blob_base.copy()
        for t, tl in enumerate(assign[core]):
            if tl is None:
                continue
            b, half, kc = tl
            qs = queries[b, half * NQS:(half + 1) * NQS]      # (128, 64)
            qTr = np.ascontiguousarray(
                qs.T.reshape(QKD, NG, 4).transpose(0, 2, 1)).reshape(QKD, NQS)
            blob[0:QKD, 64 + 128 * t:64 + 128 * (t + 1)] = qTr
            kT[:, t * KC:(t + 1) * KC] = keys[b, kc * KC:(kc + 1) * KC].T
            v = values[b, kc * KC:(kc + 1) * KC].reshape(KT, 128, VD)
            va = np.concatenate([v, np.ones((KT, 128, 1), f32)], axis=2)
            vaug[:, t * KT * 65:(t + 1) * KT * 65] = (
                va.transpose(1, 0, 2).reshape(128, KT * 65))
            kmask = (np.arange(kc * KC, (kc + 1) * KC)
                     < int(valid_lens[b])).astype(f32)
            blob[:, 192 + 128 * T + 4 * t:192 + 128 * T + 4 * (t + 1)] = (
                kmask.reshape(KT, 128).T)
        in_maps.append({
            "kT": np.ascontiguousarray(kT).astype(bf16),
            "blob": blob.astype(bf16),
            "vaug": np.ascontiguousarray(vaug).astype(bf16),
            **shared,
        })
    return T, assign, in_maps


def kernel(queries, keys, values, valid_lens, Wq, Wk, wv, _trace=False):
    T, assign, in_maps = _host_shards(
        queries, keys, values, valid_lens, Wq, Wk, wv)
    if ("nc", T) not in _cache:
        _cache[("nc", T)] = _build_nc(T)
    nc = _cache[("nc", T)]

    res = None
    for attempt in range(3):
        try:
            res = run_bass_kernel_spmd(
                nc, in_maps, core_ids=list(range(8)), trace=_trace
            )
            break
        except Exception:
            if attempt == 2:
                raise
            if attempt == 1:
                _cache.pop(("nc", T), None)
                _cache[("nc", T)] = nc = _build_nc(T)
    _cache["last_result"] = res

    # cross-shard softmax renormalization (the unshard/combine step)
    acc = np.zeros((B, NQ // NQS, NQS, VD + 1), np.float64)
    for core in range(8):
        part = res.results[core]["out"]        # (128, 65*T)
        for t, tl in enumerate(assign[core]):
            if tl is None:
                continue
            b, half, _ = tl
            acc[b, half] += part[:, t * 65:(t + 1) * 65].astype(np.float64)
    out = acc[..., :VD] / acc[..., VD:VD + 1]
    return np.ascontiguousarray(
        out.reshape(B, NQ, VD).astype(np.float32))
